# revision 26
# baseline (speedup 1.0000x reference)
"""Trainium2 Bass kernel for nn_BoundaryHead_contrast (CenterNet-style 1D NMS head).

Strategy (8 NeuronCores, pure data parallel over batch):
  - Only the *ranking* of the top-100 center logits needs high precision
    (sigmoid is monotonic; the 2e-2-relative gate on start/end is ~327 absolute).
    Masked positions (saliency < 0, ~50%) can never be selected nor suppress an
    unmasked neighbor (their center_pred is exactly 0 < any sigmoid), so the
    host compacts them away before upload.
  - Host: pack x rows at unmasked positions, transpose to [D, NKEEP], quantize
    fp8-e4m3.  W heads are a 2-level fp8 split (hi + (W-hi)*64) packed as a
    [128, 8, 6] stationary, exact to ~2^-8 relative.
  - Device: stream [128, 8, NB] fp8 tiles (4 KB+ descriptor lines, ~350 GB/s),
    DoubleRow fp8 matmuls (256-deep contraction, 0.5 cy/row) accumulating
    [6, 512] PSUM groups over the 4 chunk-pairs; ACT evacuates into a [6, NCAP]
    SBUF staging plane; one DMA returns all 6 plane rows (c/w/o x hi/lo).
  - Host: approximate center logits = hi + lo/64 (max |err| ~0.14 measured,
    margin 0.35), NMS + top-100 band selection on the approx plane, then exact
    f64 recomputation of every position within 2*margin of the approx cutoff
    (plus their window neighbors) resolves NMS decisions, the exact ranking,
    and the final start/end/score arithmetic.  Runtime asserts check the
    margin actually covers the observed error and that the selected set is
    provably complete.
"""

import numpy as np
import ml_dtypes
from contextlib import ExitStack

import concourse.bass as bass
import concourse.tile as tile
from concourse import bacc, mybir
from concourse.bass_utils import run_bass_kernel_spmd

B, L, D = 32, 8192, 1024
NCORES = 8
RPC = B // NCORES          # 4 rows per core
NROW = RPC * L             # 32768 positions per core
TOPK = 100
UNIT = 2
NEG = np.float64(-1.0e30)
MARGIN = 0.35              # logit-space bound on |approx - exact| (measured ~0.14)
LO_SCALE = np.float32(64.0)
NB = 4096                  # positions per streamed block

F8, F32, BF16 = mybir.dt.float8e4, mybir.dt.float32, mybir.dt.bfloat16

_NC_CACHE = {}


def _build_nc(ncap):
    nc = bacc.Bacc("TRN2", target_bir_lowering=False, debug=False)
    xq = nc.dram_tensor("xq", [D, ncap], F8, kind="ExternalInput").ap()
    # pre-swizzled [k, c*16+m] so the load is one 128B descriptor per partition
    st8 = nc.dram_tensor("st8", [128, 128], F8, kind="ExternalInput").ap()
    o_pl = nc.dram_tensor("o_pl", [6, ncap], BF16, kind="ExternalOutput").ap()
    DR = mybir.MatmulPerfMode.DoubleRow

    # lean ramped schedule: small first blocks for an early PE start, one small
    # last block for a fast drain, remainder merged into a mid block — every
    # extra block costs 1024 descriptors of ring/packet overhead
    if ncap >= 8192:
        up, tail = [1024, 2048], [512]
        mid_total = ncap - 3584
        mid = [NB] * (mid_total // NB)
        r = mid_total % NB
        if r:
            if mid:
                mid[-1] += r
            else:
                mid = [r]
        sched = up + mid + tail
    else:
        sched = [512] * (ncap // 512)

    with tile.TileContext(nc) as tc, ExitStack() as ctx:
        cpool = ctx.enter_context(tc.tile_pool(name="const", bufs=1))
        xpool = ctx.enter_context(tc.tile_pool(name="xin", bufs=1))
        pspool = ctx.enter_context(tc.tile_pool(name="ps", bufs=8, space="PSUM"))

        # M padded 6 -> 16: double_row ldweights needs the outermost
        # stationary step even and 16B-aligned (s3_lw_dual_fp8_restrictions)
        st_sb = cpool.tile([128, 8, 16], F8)
        nc.scalar.dma_start(st_sb[:], st8.rearrange("k (c m) -> k c m", c=8))
        stg = cpool.tile([6, ncap], BF16)

        xq_v = xq.rearrange("(c k) n -> k c n", c=8)
        # issue every x doorbell up front, each block split across BOTH HWDGE
        # rings (chunks 0-3 on sync, 4-7 on scalar): the SDMA engines
        # round-robin between rings at packet granularity, so a split block
        # completes in half the round-robin cycles and arrival order matches
        # consumption order. unique tag per block = own buffer, so the stream
        # free-runs at DMA speed instead of serializing behind PE.
        xtiles = []
        n0 = 0
        for bi, nb in enumerate(sched):
            xb = xpool.tile([128, 8, nb], F8, tag=f"xb{bi}", name=f"xb{bi}")
            nc.sync.dma_start(xb[:, 0:4, :], xq_v[:, 0:4, n0:n0 + nb])
            nc.scalar.dma_start(xb[:, 4:8, :], xq_v[:, 4:8, n0:n0 + nb])
            xtiles.append(xb)
            n0 += nb
        n0 = 0
        for bi, nb in enumerate(sched):
            xb = xtiles[bi]
            for g in range(nb // 512):
                ps = pspool.tile([16, 512], F32, tag="ps")
                for cp in range(4):
                    nc.tensor.matmul(ps[:, :], st_sb[:, 2 * cp:2 * cp + 2, :],
                                     xb[:, 2 * cp:2 * cp + 2,
                                        512 * g:512 * g + 512],
                                     start=(cp == 0), stop=(cp == 3),
                                     perf_mode=DR)
                # PSUM evacuation on the otherwise-idle Vector engine keeps
                # the scalar engine free for doorbells + plane writes
                nc.vector.tensor_copy(stg[:, n0 + 512 * g:n0 + 512 * g + 512],
                                      ps[0:6, :])
            # incremental plane writes leave only a tiny write to drain at
            # the end (one big tail DMA measured ~11 us of completion latency)
            nc.scalar.dma_start(o_pl[:, n0:n0 + nb], stg[:, n0:n0 + nb])
            n0 += nb

    nc.compile()
    return nc


def _sigmoid_like_jax(x):
    # jax.nn.sigmoid: where(x >= 0, 1/(1+exp(-x)), exp(x)/(1+exp(x))) in f32
    x = x.astype(np.float32)
    pos = x >= 0
    ex_n = np.exp(np.where(pos, -x, x).astype(np.float32)).astype(np.float32)
    out = np.where(pos,
                   (np.float32(1.0) / (np.float32(1.0) + ex_n)).astype(np.float32),
                   (ex_n / (np.float32(1.0) + ex_n)).astype(np.float32))
    return out.astype(np.float32)


def kernel(x, saliency, Wc, bc, Ww, bw, Wo, bo):
    x = np.asarray(x, dtype=np.float32)
    saliency = np.asarray(saliency, dtype=np.float32)
    Wc = np.asarray(Wc, dtype=np.float32)
    Ww = np.asarray(Ww, dtype=np.float32)
    Wo = np.asarray(Wo, dtype=np.float32)
    bc = np.float32(np.asarray(bc).reshape(-1)[0])
    bw = np.float32(np.asarray(bw).reshape(-1)[0])
    bo = np.float32(np.asarray(bo).reshape(-1)[0])

    f8 = ml_dtypes.float8_e4m3

    # ---- host prep: 2-level fp8 W stationary, mask-compacted fp8 x shards
    W3 = np.concatenate([Wc, Ww, Wo], axis=1).astype(np.float32)   # [D, 3]
    W_hi8 = W3.astype(f8)
    W_hi = W_hi8.astype(np.float32)
    W_lo8 = ((W3 - W_hi) * LO_SCALE).astype(f8)
    stw = np.zeros((D, 16), dtype=f8)                              # M padded to 16
    stw[:, 0:3] = W_hi8
    stw[:, 3:6] = W_lo8
    # pre-swizzle [c*128+k, m] -> [k, c*16+m] (one 128B line per partition)
    st8_np = np.ascontiguousarray(
        stw.reshape(8, 128, 16).transpose(1, 0, 2).reshape(128, 128))

    # global rebalance: kept positions (saliency >= 0) are split evenly across
    # the 8 cores regardless of batch row, minimizing the padded column count
    keep = saliency >= 0                                           # [B, L]
    gidx = np.where(keep.reshape(-1))[0]                           # global kept
    nk = len(gidx)
    base, extra = divmod(nk, NCORES)
    counts = [base + (1 if c < extra else 0) for c in range(NCORES)]
    offs = np.cumsum([0] + counts)
    gsplit = [gidx[offs[c]:offs[c + 1]] for c in range(NCORES)]
    ncap = int(max(512, -(-max(counts) // 512) * 512))

    key = f"nc{ncap}"
    if key not in _NC_CACHE:
        _NC_CACHE[key] = _build_nc(ncap)
    nc = _NC_CACHE[key]

    xflat = x.reshape(B * L, D)
    in_maps = []
    for c in range(NCORES):
        xq8 = np.zeros((D, ncap), dtype=f8)
        xq8[:, :counts[c]] = xflat[gsplit[c]].astype(f8).T
        in_maps.append({"xq": np.ascontiguousarray(xq8), "st8": st8_np})

    import os as _os
    trace = bool(int(_os.environ.get("KERNEL_TRACE", "0")))
    res = run_bass_kernel_spmd(nc, in_maps, core_ids=list(range(NCORES)),
                               trace=trace)
    if trace and res.exec_time_ns is not None:
        print(f"HW exec time: {res.exec_time_ns} ns")
        kernel.last_exec_time_ns = res.exec_time_ns
        kernel.last_trace = res.instructions_and_trace

    # ---- host assembly: approx center-logit grid from device planes
    cflat = np.full(B * L, NEG, dtype=np.float64)
    for c in range(NCORES):
        pl = res.results[c]["o_pl"].astype(np.float32)  # [6, ncap] bf16
        ch = (pl[0] + pl[3] / LO_SCALE)[:counts[c]]
        cflat[gsplit[c]] = ch
    cgrid = cflat.reshape(B, L)

    # approx NMS (logit space; masked = NEG never beats an unmasked sigmoid)
    pad = np.pad(cgrid, ((0, 0), (2, 2)), constant_values=NEG)
    hmax = np.max(np.stack([pad[:, i:i + L] for i in range(5)]), 0)
    sv_ap = (cgrid >= hmax) & keep

    W64 = W3.astype(np.float64)
    out = np.zeros((B, TOPK, 3), np.float32)
    for b in range(B):
        margin = MARGIN
        for attempt in range(4):
            v_ap = np.sort(cgrid[b][sv_ap[b]])[::-1]
            assert len(v_ap) >= TOPK, f"row {b}: too few approx survivors"
            cut = v_ap[TOPK - 1] - 2 * margin
            C = np.where(keep[b] & (cgrid[b] >= cut))[0]
            nb_ = np.unique(np.concatenate([C + d for d in (-2, -1, 0, 1, 2)]))
            nb_ = nb_[(nb_ >= 0) & (nb_ < L)]
            R = nb_[keep[b][nb_]]
            ex = x[b, R].astype(np.float64) @ W64                  # [nR, 3]
            err = np.abs(cgrid[b][R] - ex[:, 0]).max()
            if err >= margin / 2:
                margin *= 2
                continue
            cful = np.full(L, NEG)
            cful[R] = ex[:, 0]
            # exact NMS for candidates
            cC = cful[C]
            okm = np.ones(len(C), bool)
            for dlt in (-2, -1, 1, 2):
                j = C + dlt
                valid = (j >= 0) & (j < L)
                jj = np.clip(j, 0, L - 1)
                nbv = np.where(valid & keep[b][jj], cful[jj], NEG)
                okm &= ~(nbv > cC)
            surv = C[okm]
            cS = cful[surv]
            order = np.argsort(-cS, kind="stable")[:TOPK]
            sel = surv[order]
            csel = cS[order]
            if len(sel) < TOPK or csel[TOPK - 1] < v_ap[TOPK - 1] - margin:
                margin *= 2
                continue
            break
        else:
            raise AssertionError(f"row {b}: refinement failed to converge")

        ridx = np.searchsorted(R, sel)
        eS = ex[ridx]
        scores = _sigmoid_like_jax(eS[:, 0].astype(np.float32) + bc)
        win = np.clip((eS[:, 1].astype(np.float32) + bw).astype(np.float32),
                      np.float32(0.0), None).astype(np.float32)
        off = (eS[:, 2].astype(np.float32) + bo).astype(np.float32)
        indf = sel.astype(np.float32)
        center = np.clip((indf + off).astype(np.float32),
                         np.float32(0.0), np.float32(L - 1)).astype(np.float32)
        start = (np.clip((center - win * np.float32(0.5)).astype(np.float32),
                         np.float32(0.0), np.float32(L - 1))
                 * np.float32(UNIT)).astype(np.float32)
        end = (np.clip((center + win * np.float32(0.5)).astype(np.float32),
                       np.float32(0.0), np.float32(L - 1)) * np.float32(UNIT)
               + np.float32(UNIT)).astype(np.float32)
        out[b, :, 0] = start
        out[b, :, 1] = end
        out[b, :, 2] = scores
    return out


# revision 27
# speedup vs baseline: 1.0496x; 1.0496x over previous
"""Trainium2 Bass kernel for nn_BoundaryHead_contrast (CenterNet-style 1D NMS head).

Strategy (8 NeuronCores, pure data parallel over batch):
  - Only the *ranking* of the top-100 center logits needs high precision
    (sigmoid is monotonic; the 2e-2-relative gate on start/end is ~327 absolute).
    Masked positions (saliency < 0, ~50%) can never be selected nor suppress an
    unmasked neighbor (their center_pred is exactly 0 < any sigmoid), so the
    host compacts them away before upload.
  - Host: pack x rows at unmasked positions, transpose to [D, NKEEP], quantize
    fp8-e4m3.  W heads are a 2-level fp8 split (hi + (W-hi)*64) packed as a
    [128, 8, 6] stationary, exact to ~2^-8 relative.
  - Device: stream [128, 8, NB] fp8 tiles (4 KB+ descriptor lines, ~350 GB/s),
    DoubleRow fp8 matmuls (256-deep contraction, 0.5 cy/row) accumulating
    [6, 512] PSUM groups over the 4 chunk-pairs; ACT evacuates into a [6, NCAP]
    SBUF staging plane; one DMA returns all 6 plane rows (c/w/o x hi/lo).
  - Host: approximate center logits = hi + lo/64 (max |err| ~0.14 measured,
    margin 0.35), NMS + top-100 band selection on the approx plane, then exact
    f64 recomputation of every position within 2*margin of the approx cutoff
    (plus their window neighbors) resolves NMS decisions, the exact ranking,
    and the final start/end/score arithmetic.  Runtime asserts check the
    margin actually covers the observed error and that the selected set is
    provably complete.
"""

import numpy as np
import ml_dtypes
from contextlib import ExitStack

import concourse.bass as bass
import concourse.tile as tile
from concourse import bacc, mybir
from concourse.bass_utils import run_bass_kernel_spmd

B, L, D = 32, 8192, 1024
NCORES = 8
RPC = B // NCORES          # 4 rows per core
NROW = RPC * L             # 32768 positions per core
TOPK = 100
UNIT = 2
NEG = np.float64(-1.0e30)
MARGIN = 0.35              # logit-space bound on |approx - exact| (measured ~0.14)
LO_SCALE = np.float32(64.0)
NB = 4096                  # positions per streamed block

F8, F32, BF16 = mybir.dt.float8e4, mybir.dt.float32, mybir.dt.bfloat16

_NC_CACHE = {}


def _build_nc(ncap):
    nc = bacc.Bacc("TRN2", target_bir_lowering=False, debug=False)
    xq = nc.dram_tensor("xq", [D, ncap], F8, kind="ExternalInput").ap()
    # pre-swizzled [k, c*16+m] so the load is one 128B descriptor per partition
    st8 = nc.dram_tensor("st8", [128, 128], F8, kind="ExternalInput").ap()
    o_pl = nc.dram_tensor("o_pl", [6, ncap], BF16, kind="ExternalOutput").ap()
    DR = mybir.MatmulPerfMode.DoubleRow

    # lean ramped schedule: small first blocks for an early PE start, one small
    # last block for a fast drain, remainder merged into a mid block — every
    # extra block costs 1024 descriptors of ring/packet overhead
    if ncap >= 8192:
        up, tail = [512, 1024, 2048], [512]
        mid_total = ncap - 4096
        mid = [NB] * (mid_total // NB)
        r = mid_total % NB
        if r:
            if mid:
                mid[-1] += r
            else:
                mid = [r]
        sched = up + mid + tail
    else:
        sched = [512] * (ncap // 512)

    with tile.TileContext(nc) as tc, ExitStack() as ctx:
        cpool = ctx.enter_context(tc.tile_pool(name="const", bufs=1))
        xpool = ctx.enter_context(tc.tile_pool(name="xin", bufs=1))
        pspool = ctx.enter_context(tc.tile_pool(name="ps", bufs=8, space="PSUM"))

        # M padded 6 -> 16: double_row ldweights needs the outermost
        # stationary step even and 16B-aligned (s3_lw_dual_fp8_restrictions)
        st_sb = cpool.tile([128, 8, 16], F8)
        nc.scalar.dma_start(st_sb[:], st8.rearrange("k (c m) -> k c m", c=8))
        stg = cpool.tile([6, ncap], BF16)

        xq_v = xq.rearrange("(c k) n -> k c n", c=8)
        # issue every x doorbell up front, each block split across BOTH HWDGE
        # rings (chunks 0-3 on sync, 4-7 on scalar): the SDMA engines
        # round-robin between rings at packet granularity, so a split block
        # completes in half the round-robin cycles and arrival order matches
        # consumption order. unique tag per block = own buffer, so the stream
        # free-runs at DMA speed instead of serializing behind PE.
        xtiles = []
        n0 = 0
        for bi, nb in enumerate(sched):
            xb = xpool.tile([128, 8, nb], F8, tag=f"xb{bi}", name=f"xb{bi}")
            nc.sync.dma_start(xb[:, 0:4, :], xq_v[:, 0:4, n0:n0 + nb])
            nc.scalar.dma_start(xb[:, 4:8, :], xq_v[:, 4:8, n0:n0 + nb])
            xtiles.append(xb)
            n0 += nb
        n0 = 0
        for bi, nb in enumerate(sched):
            xb = xtiles[bi]
            for g in range(nb // 512):
                ps = pspool.tile([16, 512], F32, tag="ps")
                for cp in range(4):
                    nc.tensor.matmul(ps[:, :], st_sb[:, 2 * cp:2 * cp + 2, :],
                                     xb[:, 2 * cp:2 * cp + 2,
                                        512 * g:512 * g + 512],
                                     start=(cp == 0), stop=(cp == 3),
                                     perf_mode=DR)
                # PSUM evacuation on the otherwise-idle Vector engine keeps
                # the scalar engine free for doorbells + plane writes
                nc.vector.tensor_copy(stg[:, n0 + 512 * g:n0 + 512 * g + 512],
                                      ps[0:6, :])
            # incremental plane writes leave only a tiny write to drain at
            # the end (one big tail DMA measured ~11 us of completion latency)
            nc.scalar.dma_start(o_pl[:, n0:n0 + nb], stg[:, n0:n0 + nb])
            n0 += nb

    nc.compile()
    return nc


def _sigmoid_like_jax(x):
    # jax.nn.sigmoid: where(x >= 0, 1/(1+exp(-x)), exp(x)/(1+exp(x))) in f32
    x = x.astype(np.float32)
    pos = x >= 0
    ex_n = np.exp(np.where(pos, -x, x).astype(np.float32)).astype(np.float32)
    out = np.where(pos,
                   (np.float32(1.0) / (np.float32(1.0) + ex_n)).astype(np.float32),
                   (ex_n / (np.float32(1.0) + ex_n)).astype(np.float32))
    return out.astype(np.float32)


def kernel(x, saliency, Wc, bc, Ww, bw, Wo, bo):
    x = np.asarray(x, dtype=np.float32)
    saliency = np.asarray(saliency, dtype=np.float32)
    Wc = np.asarray(Wc, dtype=np.float32)
    Ww = np.asarray(Ww, dtype=np.float32)
    Wo = np.asarray(Wo, dtype=np.float32)
    bc = np.float32(np.asarray(bc).reshape(-1)[0])
    bw = np.float32(np.asarray(bw).reshape(-1)[0])
    bo = np.float32(np.asarray(bo).reshape(-1)[0])

    f8 = ml_dtypes.float8_e4m3

    # ---- host prep: 2-level fp8 W stationary, mask-compacted fp8 x shards
    W3 = np.concatenate([Wc, Ww, Wo], axis=1).astype(np.float32)   # [D, 3]
    W_hi8 = W3.astype(f8)
    W_hi = W_hi8.astype(np.float32)
    W_lo8 = ((W3 - W_hi) * LO_SCALE).astype(f8)
    stw = np.zeros((D, 16), dtype=f8)                              # M padded to 16
    stw[:, 0:3] = W_hi8
    stw[:, 3:6] = W_lo8
    # pre-swizzle [c*128+k, m] -> [k, c*16+m] (one 128B line per partition)
    st8_np = np.ascontiguousarray(
        stw.reshape(8, 128, 16).transpose(1, 0, 2).reshape(128, 128))

    # global rebalance: kept positions (saliency >= 0) are split evenly across
    # the 8 cores regardless of batch row, minimizing the padded column count
    keep = saliency >= 0                                           # [B, L]
    gidx = np.where(keep.reshape(-1))[0]                           # global kept
    nk = len(gidx)
    base, extra = divmod(nk, NCORES)
    counts = [base + (1 if c < extra else 0) for c in range(NCORES)]
    offs = np.cumsum([0] + counts)
    gsplit = [gidx[offs[c]:offs[c + 1]] for c in range(NCORES)]
    ncap = int(max(512, -(-max(counts) // 512) * 512))

    key = f"nc{ncap}"
    if key not in _NC_CACHE:
        _NC_CACHE[key] = _build_nc(ncap)
    nc = _NC_CACHE[key]

    xflat = x.reshape(B * L, D)
    in_maps = []
    for c in range(NCORES):
        xq8 = np.zeros((D, ncap), dtype=f8)
        xq8[:, :counts[c]] = xflat[gsplit[c]].astype(f8).T
        in_maps.append({"xq": np.ascontiguousarray(xq8), "st8": st8_np})

    import os as _os
    trace = bool(int(_os.environ.get("KERNEL_TRACE", "0")))
    res = run_bass_kernel_spmd(nc, in_maps, core_ids=list(range(NCORES)),
                               trace=trace)
    if trace and res.exec_time_ns is not None:
        print(f"HW exec time: {res.exec_time_ns} ns")
        kernel.last_exec_time_ns = res.exec_time_ns
        kernel.last_trace = res.instructions_and_trace

    # ---- host assembly: approx center-logit grid from device planes
    cflat = np.full(B * L, NEG, dtype=np.float64)
    for c in range(NCORES):
        pl = res.results[c]["o_pl"].astype(np.float32)  # [6, ncap] bf16
        ch = (pl[0] + pl[3] / LO_SCALE)[:counts[c]]
        cflat[gsplit[c]] = ch
    cgrid = cflat.reshape(B, L)

    # approx NMS (logit space; masked = NEG never beats an unmasked sigmoid)
    pad = np.pad(cgrid, ((0, 0), (2, 2)), constant_values=NEG)
    hmax = np.max(np.stack([pad[:, i:i + L] for i in range(5)]), 0)
    sv_ap = (cgrid >= hmax) & keep

    W64 = W3.astype(np.float64)
    out = np.zeros((B, TOPK, 3), np.float32)
    for b in range(B):
        margin = MARGIN
        for attempt in range(4):
            v_ap = np.sort(cgrid[b][sv_ap[b]])[::-1]
            assert len(v_ap) >= TOPK, f"row {b}: too few approx survivors"
            cut = v_ap[TOPK - 1] - 2 * margin
            C = np.where(keep[b] & (cgrid[b] >= cut))[0]
            nb_ = np.unique(np.concatenate([C + d for d in (-2, -1, 0, 1, 2)]))
            nb_ = nb_[(nb_ >= 0) & (nb_ < L)]
            R = nb_[keep[b][nb_]]
            ex = x[b, R].astype(np.float64) @ W64                  # [nR, 3]
            err = np.abs(cgrid[b][R] - ex[:, 0]).max()
            if err >= margin / 2:
                margin *= 2
                continue
            cful = np.full(L, NEG)
            cful[R] = ex[:, 0]
            # exact NMS for candidates
            cC = cful[C]
            okm = np.ones(len(C), bool)
            for dlt in (-2, -1, 1, 2):
                j = C + dlt
                valid = (j >= 0) & (j < L)
                jj = np.clip(j, 0, L - 1)
                nbv = np.where(valid & keep[b][jj], cful[jj], NEG)
                okm &= ~(nbv > cC)
            surv = C[okm]
            cS = cful[surv]
            order = np.argsort(-cS, kind="stable")[:TOPK]
            sel = surv[order]
            csel = cS[order]
            if len(sel) < TOPK or csel[TOPK - 1] < v_ap[TOPK - 1] - margin:
                margin *= 2
                continue
            break
        else:
            raise AssertionError(f"row {b}: refinement failed to converge")

        ridx = np.searchsorted(R, sel)
        eS = ex[ridx]
        scores = _sigmoid_like_jax(eS[:, 0].astype(np.float32) + bc)
        win = np.clip((eS[:, 1].astype(np.float32) + bw).astype(np.float32),
                      np.float32(0.0), None).astype(np.float32)
        off = (eS[:, 2].astype(np.float32) + bo).astype(np.float32)
        indf = sel.astype(np.float32)
        center = np.clip((indf + off).astype(np.float32),
                         np.float32(0.0), np.float32(L - 1)).astype(np.float32)
        start = (np.clip((center - win * np.float32(0.5)).astype(np.float32),
                         np.float32(0.0), np.float32(L - 1))
                 * np.float32(UNIT)).astype(np.float32)
        end = (np.clip((center + win * np.float32(0.5)).astype(np.float32),
                       np.float32(0.0), np.float32(L - 1)) * np.float32(UNIT)
               + np.float32(UNIT)).astype(np.float32)
        out[b, :, 0] = start
        out[b, :, 1] = end
        out[b, :, 2] = scores
    return out


# revision 28
# speedup vs baseline: 1.1608x; 1.1059x over previous
"""Trainium2 Bass kernel for nn_BoundaryHead_contrast (CenterNet-style 1D NMS head).

Strategy (8 NeuronCores, pure data parallel over batch):
  - Only the *ranking* of the top-100 center logits needs high precision
    (sigmoid is monotonic; the 2e-2-relative gate on start/end is ~327 absolute).
    Masked positions (saliency < 0, ~50%) can never be selected nor suppress an
    unmasked neighbor (their center_pred is exactly 0 < any sigmoid), so the
    host compacts them away before upload.
  - Host: pack x rows at unmasked positions (rebalanced evenly across cores),
    transpose to [D, NCAP], quantize fp8-e4m3.  W heads are a 2-level fp8
    split (hi + (W-hi)*64) packed as a [128, 8, 16] stationary (M padded to 16
    for the dual-fp8 ldweights alignment rule), exact to ~2^-8 relative.
  - Device: stream [128, 8, NB] fp8 tiles, each block's DMA split across both
    HWDGE rings (all doorbells issued up front, unique buffer per block, so
    the stream free-runs at ~390 GB/s aggregate); DoubleRow fp8 matmuls
    (256-deep contraction) accumulate [16, 512] PSUM groups over the 4
    chunk-pairs; the Vector engine evacuates to a bf16 [6, NCAP] staging
    plane; incremental per-block DMAs return the 6 plane rows (c/w/o hi/lo).
  - Host: approximate center logits = hi + lo/64 (max |err| ~0.14 measured,
    margin 0.35), NMS + top-100 band selection on the approx plane, then exact
    f64 recomputation of every position within 2*margin of the approx cutoff
    (plus their window neighbors) resolves NMS decisions, the exact ranking,
    and the final start/end/score arithmetic.  Runtime asserts check the
    margin actually covers the observed error and that the selected set is
    provably complete.
"""

import numpy as np
import ml_dtypes
from contextlib import ExitStack

import concourse.bass as bass
import concourse.tile as tile
from concourse import bacc, mybir
from concourse.bass_utils import run_bass_kernel_spmd

B, L, D = 32, 8192, 1024
NCORES = 8
RPC = B // NCORES          # 4 rows per core
NROW = RPC * L             # 32768 positions per core
TOPK = 100
UNIT = 2
NEG = np.float64(-1.0e30)
MARGIN = 0.35              # logit-space bound on |approx - exact| (measured ~0.14)
LO_SCALE = np.float32(64.0)
NB = 4096                  # positions per streamed block

F8, F32, BF16 = mybir.dt.float8e4, mybir.dt.float32, mybir.dt.bfloat16

_NC_CACHE = {}


def _build_nc(ncap):
    nc = bacc.Bacc("TRN2", target_bir_lowering=False, debug=False)
    xq = nc.dram_tensor("xq", [D, ncap], F8, kind="ExternalInput").ap()
    # pre-swizzled [k, c*16+m] so the load is one 128B descriptor per partition
    st8 = nc.dram_tensor("st8", [128, 128], F8, kind="ExternalInput").ap()
    o_pl = nc.dram_tensor("o_pl", [6, ncap], BF16, kind="ExternalOutput").ap()
    DR = mybir.MatmulPerfMode.DoubleRow

    # lean ramped schedule: small first blocks for an early PE start, one small
    # last block for a fast drain, remainder merged into a mid block — every
    # extra block costs 1024 descriptors of ring/packet overhead
    if ncap >= 8192:
        up, tail = [512, 1024, 2048], [512]
        mid_total = ncap - 4096
        mid = [NB] * (mid_total // NB)
        r = mid_total % NB
        if r:
            if mid:
                mid[-1] += r
            else:
                mid = [r]
        sched = up + mid + tail
    else:
        sched = [512] * (ncap // 512)

    with tile.TileContext(nc) as tc, ExitStack() as ctx:
        cpool = ctx.enter_context(tc.tile_pool(name="const", bufs=1))
        xpool = ctx.enter_context(tc.tile_pool(name="xin", bufs=1))
        pspool = ctx.enter_context(tc.tile_pool(name="ps", bufs=8, space="PSUM"))

        # M padded 6 -> 16: double_row ldweights needs the outermost
        # stationary step even and 16B-aligned (s3_lw_dual_fp8_restrictions)
        st_sb = cpool.tile([128, 8, 16], F8)
        nc.scalar.dma_start(st_sb[:], st8.rearrange("k (c m) -> k c m", c=8))
        stg = cpool.tile([6, ncap], BF16)

        xq_v = xq.rearrange("(c k) n -> k c n", c=8)
        # issue every x doorbell up front, each block split across BOTH HWDGE
        # rings (chunks 0-3 on sync, 4-7 on scalar): the SDMA engines
        # round-robin between rings at packet granularity, so a split block
        # completes in half the round-robin cycles and arrival order matches
        # consumption order. unique tag per block = own buffer, so the stream
        # free-runs at DMA speed instead of serializing behind PE.
        xtiles = []
        n0 = 0
        for bi, nb in enumerate(sched):
            xb = xpool.tile([128, 8, nb], F8, tag=f"xb{bi}", name=f"xb{bi}")
            nc.sync.dma_start(xb[:, 0:4, :], xq_v[:, 0:4, n0:n0 + nb])
            nc.scalar.dma_start(xb[:, 4:8, :], xq_v[:, 4:8, n0:n0 + nb])
            xtiles.append(xb)
            n0 += nb
        n0 = 0
        for bi, nb in enumerate(sched):
            xb = xtiles[bi]
            for g in range(nb // 512):
                ps = pspool.tile([16, 512], F32, tag="ps")
                for cp in range(4):
                    nc.tensor.matmul(ps[:, :], st_sb[:, 2 * cp:2 * cp + 2, :],
                                     xb[:, 2 * cp:2 * cp + 2,
                                        512 * g:512 * g + 512],
                                     start=(cp == 0), stop=(cp == 3),
                                     perf_mode=DR)
                # PSUM evacuation on the otherwise-idle Vector engine keeps
                # the scalar engine free for doorbells + plane writes
                nc.vector.tensor_copy(stg[:, n0 + 512 * g:n0 + 512 * g + 512],
                                      ps[0:6, :])
            # incremental plane writes leave only a tiny write to drain at
            # the end (one big tail DMA measured ~11 us of completion latency)
            nc.scalar.dma_start(o_pl[:, n0:n0 + nb], stg[:, n0:n0 + nb])
            n0 += nb

    nc.compile()
    return nc


def _sigmoid_like_jax(x):
    # jax.nn.sigmoid: where(x >= 0, 1/(1+exp(-x)), exp(x)/(1+exp(x))) in f32
    x = x.astype(np.float32)
    pos = x >= 0
    ex_n = np.exp(np.where(pos, -x, x).astype(np.float32)).astype(np.float32)
    out = np.where(pos,
                   (np.float32(1.0) / (np.float32(1.0) + ex_n)).astype(np.float32),
                   (ex_n / (np.float32(1.0) + ex_n)).astype(np.float32))
    return out.astype(np.float32)


def kernel(x, saliency, Wc, bc, Ww, bw, Wo, bo):
    x = np.asarray(x, dtype=np.float32)
    saliency = np.asarray(saliency, dtype=np.float32)
    Wc = np.asarray(Wc, dtype=np.float32)
    Ww = np.asarray(Ww, dtype=np.float32)
    Wo = np.asarray(Wo, dtype=np.float32)
    bc = np.float32(np.asarray(bc).reshape(-1)[0])
    bw = np.float32(np.asarray(bw).reshape(-1)[0])
    bo = np.float32(np.asarray(bo).reshape(-1)[0])

    f8 = ml_dtypes.float8_e4m3

    # ---- host prep: 2-level fp8 W stationary, mask-compacted fp8 x shards
    W3 = np.concatenate([Wc, Ww, Wo], axis=1).astype(np.float32)   # [D, 3]
    W_hi8 = W3.astype(f8)
    W_hi = W_hi8.astype(np.float32)
    W_lo8 = ((W3 - W_hi) * LO_SCALE).astype(f8)
    stw = np.zeros((D, 16), dtype=f8)                              # M padded to 16
    stw[:, 0:3] = W_hi8
    stw[:, 3:6] = W_lo8
    # pre-swizzle [c*128+k, m] -> [k, c*16+m] (one 128B line per partition)
    st8_np = np.ascontiguousarray(
        stw.reshape(8, 128, 16).transpose(1, 0, 2).reshape(128, 128))

    # global rebalance: kept positions (saliency >= 0) are split evenly across
    # the 8 cores regardless of batch row, minimizing the padded column count
    keep = saliency >= 0                                           # [B, L]
    gidx = np.where(keep.reshape(-1))[0]                           # global kept
    nk = len(gidx)
    base, extra = divmod(nk, NCORES)
    counts = [base + (1 if c < extra else 0) for c in range(NCORES)]
    offs = np.cumsum([0] + counts)
    gsplit = [gidx[offs[c]:offs[c + 1]] for c in range(NCORES)]
    ncap = int(max(512, -(-max(counts) // 512) * 512))

    key = f"nc{ncap}"
    if key not in _NC_CACHE:
        _NC_CACHE[key] = _build_nc(ncap)
    nc = _NC_CACHE[key]

    xflat = x.reshape(B * L, D)
    in_maps = []
    for c in range(NCORES):
        xq8 = np.zeros((D, ncap), dtype=f8)
        xq8[:, :counts[c]] = xflat[gsplit[c]].astype(f8).T
        in_maps.append({"xq": np.ascontiguousarray(xq8), "st8": st8_np})

    import os as _os
    trace = bool(int(_os.environ.get("KERNEL_TRACE", "0")))
    res = run_bass_kernel_spmd(nc, in_maps, core_ids=list(range(NCORES)),
                               trace=trace)
    if trace and res.exec_time_ns is not None:
        print(f"HW exec time: {res.exec_time_ns} ns")
        kernel.last_exec_time_ns = res.exec_time_ns
        kernel.last_trace = res.instructions_and_trace

    # ---- host assembly: approx center-logit grid from device planes
    cflat = np.full(B * L, NEG, dtype=np.float64)
    for c in range(NCORES):
        pl = res.results[c]["o_pl"].astype(np.float32)  # [6, ncap] bf16
        ch = (pl[0] + pl[3] / LO_SCALE)[:counts[c]]
        cflat[gsplit[c]] = ch
    cgrid = cflat.reshape(B, L)

    # approx NMS (logit space; masked = NEG never beats an unmasked sigmoid)
    pad = np.pad(cgrid, ((0, 0), (2, 2)), constant_values=NEG)
    hmax = np.max(np.stack([pad[:, i:i + L] for i in range(5)]), 0)
    sv_ap = (cgrid >= hmax) & keep

    W64 = W3.astype(np.float64)
    out = np.zeros((B, TOPK, 3), np.float32)
    for b in range(B):
        margin = MARGIN
        for attempt in range(4):
            v_ap = np.sort(cgrid[b][sv_ap[b]])[::-1]
            assert len(v_ap) >= TOPK, f"row {b}: too few approx survivors"
            cut = v_ap[TOPK - 1] - 2 * margin
            C = np.where(keep[b] & (cgrid[b] >= cut))[0]
            nb_ = np.unique(np.concatenate([C + d for d in (-2, -1, 0, 1, 2)]))
            nb_ = nb_[(nb_ >= 0) & (nb_ < L)]
            R = nb_[keep[b][nb_]]
            ex = x[b, R].astype(np.float64) @ W64                  # [nR, 3]
            err = np.abs(cgrid[b][R] - ex[:, 0]).max()
            if err >= margin / 2:
                margin *= 2
                continue
            cful = np.full(L, NEG)
            cful[R] = ex[:, 0]
            # exact NMS for candidates
            cC = cful[C]
            okm = np.ones(len(C), bool)
            for dlt in (-2, -1, 1, 2):
                j = C + dlt
                valid = (j >= 0) & (j < L)
                jj = np.clip(j, 0, L - 1)
                nbv = np.where(valid & keep[b][jj], cful[jj], NEG)
                okm &= ~(nbv > cC)
            surv = C[okm]
            cS = cful[surv]
            order = np.argsort(-cS, kind="stable")[:TOPK]
            sel = surv[order]
            csel = cS[order]
            if len(sel) < TOPK or csel[TOPK - 1] < v_ap[TOPK - 1] - margin:
                margin *= 2
                continue
            break
        else:
            raise AssertionError(f"row {b}: refinement failed to converge")

        ridx = np.searchsorted(R, sel)
        eS = ex[ridx]
        scores = _sigmoid_like_jax(eS[:, 0].astype(np.float32) + bc)
        win = np.clip((eS[:, 1].astype(np.float32) + bw).astype(np.float32),
                      np.float32(0.0), None).astype(np.float32)
        off = (eS[:, 2].astype(np.float32) + bo).astype(np.float32)
        indf = sel.astype(np.float32)
        center = np.clip((indf + off).astype(np.float32),
                         np.float32(0.0), np.float32(L - 1)).astype(np.float32)
        start = (np.clip((center - win * np.float32(0.5)).astype(np.float32),
                         np.float32(0.0), np.float32(L - 1))
                 * np.float32(UNIT)).astype(np.float32)
        end = (np.clip((center + win * np.float32(0.5)).astype(np.float32),
                       np.float32(0.0), np.float32(L - 1)) * np.float32(UNIT)
               + np.float32(UNIT)).astype(np.float32)
        out[b, :, 0] = start
        out[b, :, 1] = end
        out[b, :, 2] = scores
    return out
